# revision 14
# baseline (speedup 1.0000x reference)
"""DNC (Differentiable Neural Computer) scan kernel for Trainium2.

Sharding: data-parallel over batch B=32 across 8 NeuronCores (4 examples/core).
Per core, a fully-unrolled T-step scan. Layout puts the N=128 memory slots in
the SBUF partition dimension.

Row-broadcasts (column-layout [128, b] -> [128, b, 128] with value j on free
position j) are built as  allreduce_partitions( X[p,b] * eye[p,j] ):  the
eye-mask multiply runs on the Scalar engine (activation Identity with a
per-partition scale), the partition reduction on GPSIMD partition_all_reduce.

Allocation weighting (stable argsort + shifted cumprod + scatter in the
reference) is computed sort-free as a masked product:
    alloc[i] = (1-u[i]) * prod_j ( 1 + [u~_j < u~_i] * (u_j - 1) )
where u~ = u + slot_index*1e-7 reproduces the stable-sort index tie-break.

The content-weight softmax uses  exp(cos * softplus(z)) = (1+e^z)^cos,
computed with the DVE pow ALU op, so no exp/ln activation tables are needed;
the single activation table set is sigmoid_and_others (Sigmoid/Tanh native).
"""

import sys
from contextlib import ExitStack

import ml_dtypes
import numpy as np

BFNP = ml_dtypes.bfloat16

sys.path.insert(0, "/opt/trn_rl_repo")

import concourse.bass as bass  # noqa: E402
import concourse.bacc as bacc  # noqa: E402
import concourse.bass_isa as bass_isa  # noqa: E402
import concourse.tile as tile  # noqa: E402
from concourse import mybir  # noqa: E402
from concourse import bass_utils  # noqa: E402
from concourse.hw_specs import get_activation_tables  # noqa: E402

AF = mybir.ActivationFunctionType
OP = mybir.AluOpType
AX = mybir.AxisListType
RED = bass_isa.ReduceOp
F32 = mybir.dt.float32
BF16 = mybir.dt.bfloat16

B, T, I, O, H = 32, 64, 64, 64, 512
N, W = 128, 64
IF = 3 * W + 6
NCORES = 8
BB = B // NCORES  # 4 examples per core
NMB = 16          # gate-dim m-blocks (2048/128)
NKH = 4           # h k-chunks (512/128)
SC6 = [128, 129, 194, 195, 196, 197]  # live dims of itf block >=128

# column blocks inside the shared small-psum tile [128, 8, BB]
S_ITF = 0    # 0:2  itfT blocks
S_BW = 2     # 2    bw
S_RV = 3     # 3    rvec (rows 0:64)
S_SS = 4     # partition-sum rows at [0:1, 4..6, :]
S_CS = 5
S_WS = 6


class _Bacc(bacc.Bacc):
    """Bacc with the activation-table list reordered so one set
    (sigmoid_and_others: Sigmoid/Tanh/Identity/Copy/Square) covers every
    activation in the kernel -> a single table load, no mid-kernel reloads."""

    def insert_act_table_loads(self):
        has_activation = any(
            isinstance(i, mybir.InstActivation)
            for b in self.main_func.blocks
            for i in b.instructions
        )
        if not has_activation:
            return
        tables = [
            (name, (s if name == "sigmoid_and_others" else type(s)()))
            for name, s in get_activation_tables(self.m.arch).items()
        ]
        import bass_rust as _bass_rust
        _bass_rust.insert_act_table_loads(self, tables)


def _mid_bcast(ap2d, count):
    """[P, F] -> [P, count, F] broadcasting along a new middle dim."""
    a = ap2d.ap
    assert len(a) == 2, a
    return bass.AP(tensor=ap2d.tensor, offset=ap2d.offset,
                   ap=[a[0], [0, count], a[1]])


def build(T_steps=T, dbg=False):
    nc = _Bacc("TRN2", target_bir_lowering=False, debug=False)

    def din(name, shape, dt=F32):
        return nc.dram_tensor(name, shape, dt, kind="ExternalInput").ap()

    xt = din("xt", [I, T_steps, BB], BF16)
    wx = din("wx", [I, NMB, 128], BF16)
    wgr = din("wgr", [I, NMB, 128], BF16)
    wgh = din("wgh", [128, NKH, NMB, 128], BF16)
    wif = din("wif", [128, NKH, 2, 128], BF16)
    wout = din("wout", [128, 5, O], BF16)
    bg = din("bg", [128, NMB])
    bif = din("bif", [128, 2])
    bout = din("bout", [O, 1])
    pert = din("pert", [128, 1])
    eye = din("eye", [128, 128])
    eyem = din("eyem", [128, 128])
    pm1 = din("pm1", [128, 128])
    econ = din("econ", [128, 1])
    onesc = din("onesc", [128, 1])
    eps8 = din("eps8", [128, 1])
    half = din("half", [128, 1])

    yT = nc.dram_tensor("yT", [O, T_steps, BB], F32, kind="ExternalOutput").ap()

    dbg_outs = {}
    if dbg:
        for nm, shp in [
            ("d_h", [T_steps, 128, NKH, BB]),
            ("d_rv", [T_steps, W, BB]),
        ]:
            dbg_outs[nm] = nc.dram_tensor(nm, shp, BF16,
                                          kind="ExternalOutput").ap()
        for nm, shp in [
            ("d_gates", [T_steps, 128, NMB, BB]),
            ("d_itf", [T_steps, 128, 2, BB]),
            ("d_alloc", [T_steps, 128, BB]),
            ("d_ww", [T_steps, 128, BB]),
            ("d_usage", [T_steps, 128, BB]),
            ("d_mem", [T_steps, 128, BB, W]),
            ("d_link", [T_steps, 128, BB, 128]),
            ("d_wts", [T_steps, 128, BB]),
            ("d_prec", [T_steps, 128, BB]),
        ]:
            dbg_outs[nm] = nc.dram_tensor(nm, shp, F32, kind="ExternalOutput").ap()

    with tile.TileContext(nc) as tc, ExitStack() as ctx:
        state = ctx.enter_context(tc.tile_pool(name="state", bufs=1))
        scr = ctx.enter_context(tc.tile_pool(name="scr", bufs=2))
        p_gates = ctx.enter_context(
            tc.tile_pool(name="p_gates", bufs=1, space="PSUM"))
        p_small = ctx.enter_context(
            tc.tile_pool(name="p_small", bufs=1, space="PSUM"))
        p_ulb = ctx.enter_context(
            tc.tile_pool(name="p_ulb", bufs=1, space="PSUM"))

        def load(name, ap_dram, shape, dt=F32):
            t = state.tile(shape, dt, name=name)
            nc.sync.dma_start(out=t, in_=ap_dram)
            return t

        XT = load("XT", xt, [I, T_steps, BB], BF16)
        WX = load("WX", wx, [I, NMB, 128], BF16)
        WGR = load("WGR", wgr, [I, NMB, 128], BF16)
        WGH = load("WGH", wgh, [128, NKH, NMB, 128], BF16)
        WIF = load("WIF", wif, [128, NKH, 2, 128], BF16)
        WOUT = load("WOUT", wout, [128, 5, O], BF16)
        BG = load("BG", bg, [128, NMB])
        BIF = load("BIF", bif, [128, 2])
        BOUT = load("BOUT", bout, [O, 1])
        PERT = load("PERT", pert, [128, 1])
        EYE = load("EYE", eye, [128, 128])
        EYEM = load("EYEM", eyem, [128, 128])
        PM1 = load("PM1", pm1, [128, 128])
        ECON = load("ECON", econ, [128, 1])
        ONESC = load("ONESC", onesc, [128, 1])
        EPS8 = load("EPS8", eps8, [128, 1])
        HALF = load("HALF", half, [128, 1])

        # ---- persistent state ----
        GX = state.tile([128, NMB, T_steps, BB], F32, name="GX")
        HR = state.tile([128, 5, T_steps + 1, BB], BF16, name="HR")
        MEM = state.tile([128, BB, W], F32, name="MEM")
        MEMB = state.tile([128, BB, W], BF16, name="MEMB")
        LINK2 = state.tile([128, 2, BB, 128], F32, name="LINK2")
        LINKNB = state.tile([128, BB, 128], BF16, name="LINKNB")
        USAGE = state.tile([128, BB], F32, name="USAGE")
        PREC = state.tile([128, BB], F32, name="PREC")
        CT = state.tile([128, NKH, BB], F32, name="CT")
        WTS2 = state.tile([128, 2, BB], F32, name="WTS2")
        WTS16 = state.tile([128, 2, BB], BF16, name="WTS16")

        for tl in (HR, MEM, LINK2, USAGE, PREC, CT, WTS2, WTS16):
            nc.vector.memset(tl, 0.0)

        def rowbcast(dst3, src_col, ncols, mask2d, mask_tile):
            """dst3[i, b, j] = src_col[j, b] for j < ncols."""
            nc.vector.tensor_tensor(
                mask_tile, src_col.broadcast_to([128, BB, ncols]),
                _mid_bcast(mask2d, BB), OP.mult)
            nc.gpsimd.partition_all_reduce(
                dst3.rearrange("p b j -> p (b j)"),
                mask_tile.rearrange("p b j -> p (b j)"),
                channels=128, reduce_op=RED.add)

        # ---- pre-GEMM: GX[g,(t,b)] = W_ih[:, :64] @ x + (b_ih + b_hh) ----
        for mb in range(NMB):
            gx_ps = p_ulb.tile([128, T_steps * BB], F32, tag="ulb",
                               name="gx_ps")
            nc.tensor.matmul(gx_ps, WX[:, mb, :],
                             XT.rearrange("i t b -> i (t b)"),
                             start=True, stop=True)
            nc.vector.tensor_scalar(
                out=GX[:, mb, :, :].rearrange("p t b -> p (t b)"), in0=gx_ps,
                scalar1=BG[:, mb:mb + 1], scalar2=None, op0=OP.add)

        # =================== the scan ===================
        for t in range(T_steps):
            rw = WTS2[:, t % 2, :]
            wts_new = WTS2[:, (t + 1) % 2, :]
            LINK = LINK2[:, t % 2, :, :]
            LINKN = LINK2[:, (t + 1) % 2, :, :]

            # ---- E-early: allocation weighting (only needs usage) ----
            ucol = scr.tile([128, BB], F32, name="ucol")  # u~ column
            nc.vector.tensor_scalar(out=ucol, in0=USAGE, scalar1=PERT,
                                    scalar2=None, op0=OP.add)
            ubc = scr.tile([128, BB, 128], F32, name="ubc")      # u~ row-bcast
            ubcm = scr.tile([128, BB, 128], F32, name="ubcm")
            rowbcast(ubc, ucol, 128, EYE, ubcm)
            # um1[i,b,j] = u_j - 1   (un-perturb + shift via constant)
            um1 = scr.tile([128, BB, 128], F32, name="um1")
            nc.vector.tensor_tensor(um1, ubc, _mid_bcast(PM1, BB), OP.add)
            # masked terms:  m3 = [u~_j < u~_i] * (u_j - 1);  m4 = m3 + 1
            m4 = scr.tile([128, BB, 128], F32, name="m4")
            for b in range(BB):
                nc.vector.scalar_tensor_tensor(
                    out=m4[:, b, :], in0=ubc[:, b, :],
                    scalar=ucol[:, b:b + 1], in1=um1[:, b, :],
                    op0=OP.is_lt, op1=OP.mult)
            nc.scalar.activation(out=m4.rearrange("p b j -> p (b j)"),
                                 in_=m4.rearrange("p b j -> p (b j)"),
                                 func=AF.Identity, bias=1.0)
            pprod = scr.tile([128, BB], F32, name="pprod")
            nc.vector.tensor_reduce(out=pprod, in_=m4, axis=AX.X, op=OP.mult)
            onemu = scr.tile([128, BB], F32, name="onemu")
            nc.vector.tensor_scalar(out=onemu, in0=USAGE, scalar1=-1.0,
                                    scalar2=1.0, op0=OP.mult, op1=OP.add)
            alloc = scr.tile([128, BB], F32, name="alloc")
            nc.vector.tensor_mul(alloc, onemu, pprod)
            allocbc = scr.tile([128, BB, 128], F32, name="allocbc")
            allocbm = scr.tile([128, BB, 128], F32, name="allocbm")
            rowbcast(allocbc, alloc, 128, EYE, allocbm)
            salloc = scr.tile([128, BB], F32, name="salloc")
            nc.gpsimd.partition_all_reduce(salloc, alloc, channels=128,
                                           reduce_op=RED.add)

            # ---- broadcast rw rows early (for fw) ----
            rwb = scr.tile([128, BB, 128], F32, name="rwb")
            rwbm = scr.tile([128, BB, 128], F32, name="rwbm")
            rowbcast(rwb, rw, 128, EYE, rwbm)

            # ---- A: gates = GX[t] + W_cat @ [rvec; h] ----
            g_ps = p_gates.tile([128, NMB, BB], F32, tag="gates", name="g_ps")
            for mb in range(NMB):
                for c in range(5):
                    rhs = HR[0:64, 0, t, :] if c == 0 else HR[:, c, t, :]
                    lhsT = WGR[:, mb, :] if c == 0 else WGH[:, c - 1, mb, :]
                    nc.tensor.matmul(g_ps[:, mb, :], lhsT, rhs,
                                     start=(c == 0), stop=(c == 4))
            gates = scr.tile([128, NMB, BB], F32, name="gates")
            nc.vector.tensor_add(gates, g_ps, GX[:, :, t, :])

            # ---- B: LSTM cell (gate order i,f,o,g after host repack) ----
            act = scr.tile([128, NMB, BB], F32, name="act")
            nc.scalar.activation(out=act[:, 0:12, :], in_=gates[:, 0:12, :],
                                 func=AF.Sigmoid)
            nc.scalar.activation(out=act[:, 12:16, :], in_=gates[:, 12:16, :],
                                 func=AF.Tanh)
            c1 = scr.tile([128, NKH, BB], F32, name="c1")
            nc.vector.tensor_mul(c1, act[:, 4:8, :], CT)
            c2 = scr.tile([128, NKH, BB], F32, name="c2")
            nc.vector.tensor_mul(c2, act[:, 0:4, :], act[:, 12:16, :])
            nc.vector.tensor_add(CT, c1, c2)
            tanc = scr.tile([128, NKH, BB], F32, name="tanc")
            nc.scalar.activation(out=tanc, in_=CT, func=AF.Tanh)
            nc.vector.tensor_mul(HR[:, 1:5, t + 1, :], act[:, 8:12, :], tanc)

            SMALL = p_small.tile([128, 8, BB], F32, tag="small", name="SMALL")

            # ---- C: interface itfT = W_if @ h + b_if ----
            for blk in range(2):
                for c in range(NKH):
                    nc.tensor.matmul(
                        SMALL[:, S_ITF + blk, :], WIF[:, c, blk, :],
                        HR[:, 1 + c, t + 1, :],
                        start=(c == 0), stop=(c == NKH - 1))
            itf = scr.tile([128, 2, BB], F32, name="itf")
            nc.vector.tensor_add(itf, SMALL[:, S_ITF:S_ITF + 2, :],
                                 BIF.broadcast_to([128, 2, BB]))

            # ---- D: broadcast interface rows; derive per-example scalars ----
            ewk = scr.tile([128, BB, 128], F32, name="ewk")
            ewkm = scr.tile([128, BB, 128], F32, name="ewkm")
            rowbcast(ewk, itf[:, 0, :], 128, EYE, ewkm)
            # block1 live dims packed at m=0..5: wg, ag, z(=rstr src), rmode*3
            sc1 = scr.tile([128, BB, 6], F32, name="sc1")
            sc1m = scr.tile([128, BB, 6], F32, name="sc1m")
            rowbcast(sc1, itf[:, 1, :], 6, EYE[:, 0:6], sc1m)

            erase_bc = scr.tile([128, BB, W], F32, name="erase_bc")
            nc.scalar.activation(out=erase_bc, in_=ewk[:, :, 0:64],
                                 func=AF.Sigmoid)
            sgb = scr.tile([128, BB, 2], F32, name="sgb")
            nc.scalar.activation(out=sgb, in_=sc1[:, :, 0:2], func=AF.Sigmoid)
            wgag = scr.tile([128, BB], F32, name="wgag")
            nc.vector.tensor_mul(wgag, sgb[:, :, 0], sgb[:, :, 1])
            # rmode = softmax(itf[:, -3:]) via pow(e, x)
            ex3 = scr.tile([128, BB, 3], F32, name="ex3")
            nc.gpsimd.tensor_tensor(
                ex3, bass.AP(tensor=ECON.tensor, offset=ECON.offset,
                             ap=[ECON.ap[0], [0, BB], [0, 3]]),
                sc1[:, :, 3:6], OP.pow)
            ex3s = scr.tile([128, BB], F32, name="ex3s")
            nc.vector.tensor_reduce(out=ex3s, in_=ex3, axis=AX.X, op=OP.add)
            nc.vector.reciprocal(out=ex3s, in_=ex3s)
            rmb = scr.tile([128, BB, 3], F32, name="rmb")
            nc.vector.tensor_mul(rmb, ex3, ex3s.broadcast_to([128, BB, 3]))
            # cw base = 1 + e^z ; exponent needs 1/(kn*mn)
            ez = scr.tile([128, BB], F32, name="ez")
            nc.gpsimd.tensor_tensor(
                ez, bass.AP(tensor=ECON.tensor, offset=ECON.offset,
                            ap=[ECON.ap[0], [0, BB]]),
                sc1[:, :, 2], OP.pow)
            base = scr.tile([128, BB], F32, name="base")
            nc.scalar.activation(out=base, in_=ez, func=AF.Identity, bias=1.0)
            ksq = scr.tile([128, BB, W], F32, name="ksq")
            nc.scalar.activation(out=ksq, in_=ewk[:, :, 0:64], func=AF.Square)
            kn2 = scr.tile([128, BB], F32, name="kn2")
            nc.vector.tensor_reduce(out=kn2, in_=ksq, axis=AX.X, op=OP.add)
            kn = scr.tile([128, BB], F32, name="kn")
            nc.gpsimd.tensor_tensor(kn, kn2, HALF.broadcast_to([128, BB]),
                                    OP.pow)
            nc.scalar.activation(out=kn, in_=kn, func=AF.Identity, bias=EPS8)

            # ---- ww = wg*ag*alloc ----
            ww = scr.tile([128, BB], F32, name="ww")
            nc.vector.tensor_mul(ww, alloc, wgag)

            # ---- F: memory write + usage ----
            t1 = scr.tile([128, BB, W], F32, name="t1")
            nc.vector.tensor_mul(t1, erase_bc, ww.broadcast_to([128, BB, W]))
            m1 = scr.tile([128, BB, W], F32, name="m1")
            nc.vector.scalar_tensor_tensor(out=m1, in0=t1, scalar=-1.0,
                                           in1=MEM, op0=OP.mult, op1=OP.mult)
            nc.vector.tensor_add(MEM, MEM, m1)
            wv = scr.tile([128, BB, W], F32, name="wv")
            nc.vector.tensor_mul(wv, ewk[:, :, 64:128],
                                 ww.broadcast_to([128, BB, W]))
            nc.vector.tensor_add(MEM, MEM, wv)
            nc.scalar.activation(out=MEMB, in_=MEM, func=AF.Copy)
            uw = scr.tile([128, BB], F32, name="uw")
            nc.vector.tensor_mul(uw, onemu, ww)
            nc.vector.tensor_add(USAGE, USAGE, uw)

            # ---- G: link update ----
            # wwr[i,b,j] = ww[j,b] = allocbc * wgag  (no bcast on the chain)
            wwr = scr.tile([128, BB, 128], F32, name="wwr")
            nc.vector.tensor_mul(wwr, allocbc,
                                 wgag.broadcast_to([128, BB, 128]))
            onemw = scr.tile([128, BB], F32, name="onemw")
            nc.vector.tensor_scalar(out=onemw, in0=ww, scalar1=-1.0,
                                    scalar2=1.0, op0=OP.mult, op1=OP.add)
            decay = scr.tile([128, BB, 128], F32, name="decay")
            nc.vector.tensor_tensor(
                decay, onemw.broadcast_to([128, BB, 128]), wwr, OP.subtract)
            ld = scr.tile([128, BB, 128], F32, name="ld")
            nc.vector.tensor_mul(ld, LINK, decay)
            t2a = scr.tile([128, BB, 128], F32, name="t2a")
            nc.gpsimd.tensor_mul(t2a, wwr, PREC.broadcast_to([128, BB, 128]))
            t2m = scr.tile([128, BB, 128], F32, name="t2m")
            nc.gpsimd.tensor_tensor(t2m, t2a, _mid_bcast(EYEM, BB), OP.mult)
            nc.vector.tensor_add(LINKN, ld, t2m)
            nc.scalar.activation(out=LINKNB, in_=LINKN, func=AF.Copy)
            # prec = (1 - wgag*sum(alloc)) * prec + ww
            sww = scr.tile([128, BB], F32, name="sww")
            nc.vector.tensor_mul(sww, salloc, wgag)
            oms = scr.tile([128, BB], F32, name="oms")
            nc.vector.tensor_scalar(out=oms, in0=sww, scalar1=-1.0,
                                    scalar2=1.0, op0=OP.mult, op1=OP.add)
            nc.vector.tensor_mul(PREC, PREC, oms)
            nc.vector.tensor_add(PREC, PREC, ww)

            # ---- H: read ----
            rw16 = WTS16[:, t % 2, :]
            for b in range(BB):
                nc.tensor.matmul(SMALL[:, S_BW, b:b + 1], LINKNB[:, b, :],
                                 rw16[:, b:b + 1], start=True, stop=True)
            fw = scr.tile([128, BB], F32, name="fw")
            junkH = scr.tile([128, BB, 128], F32, name="junkH")
            for b in range(BB):
                nc.vector.scalar_tensor_tensor(
                    out=junkH[:, b, :], in0=LINKN[:, b, :], scalar=1.0,
                    in1=rwb[:, b, :], op0=OP.mult, op1=OP.mult,
                    accum_out=fw[:, b:b + 1])
            msq = scr.tile([128, BB, W], F32, name="msq")
            nc.scalar.activation(out=msq, in_=MEM, func=AF.Square)
            mn2 = scr.tile([128, BB], F32, name="mn2")
            nc.vector.tensor_reduce(out=mn2, in_=msq, axis=AX.X, op=OP.add)
            dsq = scr.tile([128, BB, W], F32, name="dsq")
            nc.vector.tensor_mul(dsq, MEM, ewk[:, :, 0:64])
            dotv = scr.tile([128, BB], F32, name="dotv")
            nc.vector.tensor_reduce(out=dotv, in_=dsq, axis=AX.X, op=OP.add)
            mn = scr.tile([128, BB], F32, name="mn")
            nc.gpsimd.tensor_tensor(mn, mn2, HALF.broadcast_to([128, BB]),
                                    OP.pow)
            nc.scalar.activation(out=mn, in_=mn, func=AF.Identity, bias=EPS8)
            rkn = scr.tile([128, BB], F32, name="rkn")
            nc.vector.reciprocal(out=rkn, in_=kn)
            rmn = scr.tile([128, BB], F32, name="rmn")
            nc.vector.reciprocal(out=rmn, in_=mn)
            e1 = scr.tile([128, BB], F32, name="e1")
            nc.vector.tensor_mul(e1, dotv, rkn)
            expv = scr.tile([128, BB], F32, name="expv")
            nc.vector.tensor_mul(expv, e1, rmn)
            cwn = scr.tile([128, BB], F32, name="cwn")
            nc.gpsimd.tensor_tensor(cwn, base, expv, OP.pow)
            csb = scr.tile([128, BB], F32, name="csb")
            nc.gpsimd.partition_all_reduce(csb, cwn, channels=128,
                                           reduce_op=RED.add)
            nc.vector.reciprocal(out=csb, in_=csb)
            wtsu = scr.tile([128, BB], F32, name="wtsu")
            cw = scr.tile([128, BB], F32, name="cw")
            nc.vector.tensor_mul(cw, cwn, csb)
            nc.vector.tensor_mul(cw, cw, rmb[:, :, 2])
            nc.vector.tensor_mul(wtsu, SMALL[:, S_BW, :], rmb[:, :, 0])
            nc.vector.tensor_mul(fw, fw, rmb[:, :, 1])
            nc.vector.tensor_add(wtsu, wtsu, fw)
            nc.vector.tensor_add(wtsu, wtsu, cw)
            nc.scalar.activation(out=wtsu, in_=wtsu, func=AF.Identity,
                                 bias=EPS8)
            wtsu16 = scr.tile([128, BB], BF16, name="wtsu16")
            nc.scalar.activation(out=wtsu16, in_=wtsu, func=AF.Copy)
            # rvec on pre-normalized weights; scale afterwards
            for b in range(BB):
                nc.tensor.matmul(SMALL[0:64, S_RV, b:b + 1], MEMB[:, b, :],
                                 wtsu16[:, b:b + 1], start=True, stop=True)
            wsb = scr.tile([128, BB], F32, name="wsb")
            nc.gpsimd.partition_all_reduce(wsb, wtsu, channels=128,
                                           reduce_op=RED.add)
            nc.vector.reciprocal(out=wsb, in_=wsb)
            nc.vector.tensor_mul(wts_new, wtsu, wsb)
            nc.vector.tensor_mul(WTS16[:, (t + 1) % 2, :], wtsu, wsb)
            nc.vector.tensor_mul(HR[0:64, 0, t + 1, :],
                                 SMALL[0:64, S_RV, :], wsb[0:64, :])

            if dbg:
                nc.sync.dma_start(out=dbg_outs["d_gates"][t], in_=gates)
                nc.sync.dma_start(out=dbg_outs["d_itf"][t], in_=itf)
                nc.sync.dma_start(out=dbg_outs["d_alloc"][t], in_=alloc)
                nc.sync.dma_start(out=dbg_outs["d_ww"][t], in_=ww)
                nc.sync.dma_start(out=dbg_outs["d_usage"][t], in_=USAGE)
                nc.sync.dma_start(out=dbg_outs["d_mem"][t], in_=MEM)
                nc.sync.dma_start(out=dbg_outs["d_link"][t], in_=LINKN)
                nc.sync.dma_start(out=dbg_outs["d_wts"][t], in_=wts_new)
                nc.sync.dma_start(out=dbg_outs["d_h"][t],
                                  in_=HR[:, 1:5, t + 1, :])
                nc.sync.dma_start(out=dbg_outs["d_rv"][t],
                                  in_=HR[0:64, 0, t + 1, :])
                nc.sync.dma_start(out=dbg_outs["d_prec"][t], in_=PREC)

        # ---- post-GEMM: y = W_out @ [h; rvec] + b_out ----
        y_ps = p_ulb.tile([O, T_steps * BB], F32, tag="ulb", name="y_ps")
        for c in range(5):
            if c == 0:
                rhs = HR[0:64, 0, 1:T_steps + 1, :].rearrange(
                    "p t b -> p (t b)")
                nc.tensor.matmul(y_ps, WOUT[0:64, 0, :], rhs,
                                 start=True, stop=False)
            else:
                rhs = HR[:, c, 1:T_steps + 1, :].rearrange("p t b -> p (t b)")
                nc.tensor.matmul(y_ps, WOUT[:, c, :], rhs,
                                 start=False, stop=(c == 4))
        y_sb = state.tile([O, T_steps * BB], F32, name="y_sb")
        nc.vector.tensor_scalar(out=y_sb, in0=y_ps, scalar1=BOUT,
                                scalar2=None, op0=OP.add)
        nc.sync.dma_start(out=yT, in_=y_sb.rearrange("o (t b) -> o t b", b=BB))

    nc.finalize()
    return nc


def prep_inputs(x, W_ih, W_hh, b_ih, b_hh, W_if, b_if, W_out, b_out,
                T_steps=T):
    """Host-side layout packing of full inputs into per-core in_maps."""
    x = np.asarray(x, np.float32)
    W_ih = np.asarray(W_ih, np.float32)
    W_hh = np.asarray(W_hh, np.float32)
    W_if_ = np.asarray(W_if, np.float32)
    W_out_ = np.asarray(W_out, np.float32)
    b_ih = np.asarray(b_ih, np.float32)
    b_hh = np.asarray(b_hh, np.float32)
    b_if_ = np.asarray(b_if, np.float32)
    b_out_ = np.asarray(b_out, np.float32)

    # gate-dim m-block permutation: source order i,f,g,o -> kernel i,f,o,g
    gperm = [0, 1, 2, 3, 4, 5, 6, 7, 12, 13, 14, 15, 8, 9, 10, 11]
    wx = np.ascontiguousarray(
        W_ih[:, :64].T.reshape(I, NMB, 128)[:, gperm, :])
    wgr = np.ascontiguousarray(
        W_ih[:, 64:128].T.reshape(I, NMB, 128)[:, gperm, :])
    wgh = np.ascontiguousarray(
        W_hh.T.reshape(NKH, 128, NMB, 128).transpose(1, 0, 2, 3)[
            :, :, gperm, :])
    wifT = W_if_.T.reshape(NKH, 128, IF).transpose(1, 0, 2)  # [k, c, d]
    wif = np.zeros((128, NKH, 2, 128), np.float32)
    wif[:, :, 0, :] = wifT[:, :, 0:128]
    wif[:, :, 1, 0:6] = wifT[:, :, SC6]
    wout = np.zeros((128, 5, O), np.float32)
    wout[0:64, 0, :] = W_out_[:, 512:576].T
    wout[:, 1:5, :] = W_out_[:, 0:512].T.reshape(NKH, 128, O).transpose(
        1, 0, 2)
    bgv = np.ascontiguousarray((b_ih + b_hh).reshape(NMB, 128)[gperm, :].T)
    bifv = np.zeros((128, 2), np.float32)
    bifv[:, 0] = b_if_[0:128]
    bifv[0:6, 1] = b_if_[SC6]
    boutv = np.ascontiguousarray(b_out_.reshape(O, 1))

    jj = np.arange(128, dtype=np.float64)
    common = {
        "wx": wx.astype(BFNP), "wgr": wgr.astype(BFNP),
        "wgh": wgh.astype(BFNP), "wif": wif.astype(BFNP),
        "wout": wout.astype(BFNP),
        "bg": bgv, "bif": bifv, "bout": boutv,
        "pert": (np.arange(128, dtype=np.float32) * 1e-7).reshape(128, 1),
        "eye": np.eye(128, dtype=np.float32),
        "eyem": (1.0 - np.eye(128)).astype(np.float32),
        "pm1": np.tile((-(1.0 + jj * 1e-7)).astype(np.float32), (128, 1)),
        "econ": np.full((128, 1), np.e, np.float32),
        "onesc": np.ones((128, 1), np.float32),
        "eps8": np.full((128, 1), 1e-8, np.float32),
        "half": np.full((128, 1), 0.5, np.float32),
    }
    in_maps = []
    for core in range(NCORES):
        xc = x[core * BB:(core + 1) * BB, :T_steps, :]
        m = dict(common)
        m["xt"] = np.ascontiguousarray(xc.transpose(2, 1, 0)).astype(BFNP)
        in_maps.append(m)
    return in_maps


_NC_CACHE = {}


def run(inputs, T_steps=T, dbg=False, trace=False):
    key = (T_steps, dbg)
    if key not in _NC_CACHE:
        _NC_CACHE[key] = build(T_steps, dbg)
    nc = _NC_CACHE[key]
    in_maps = prep_inputs(**inputs, T_steps=T_steps)
    res = bass_utils.run_bass_kernel_spmd(
        nc, in_maps, core_ids=list(range(NCORES)), trace=trace)
    outs = [np.asarray(r["yT"]).transpose(2, 1, 0) for r in res.results]
    return np.concatenate(outs, axis=0), res


def kernel(**inputs):
    y, _ = run(inputs, T_steps=T, dbg=False)
    return np.ascontiguousarray(y).astype(np.float32)

